# revision 1
# baseline (speedup 1.0000x reference)
"""HBMP (3-branch LSTM + BiLSTM + global max pool) Trainium2 kernel.

Model (B=64, T=512, E=300, H=512, NB=3 branches):
  per branch: h1 = LSTM(x); hf = LSTM(h1); hb = rev(LSTM(rev(h1)))
  emb = maxpool_T(concat([hf, hb], -1));  out = concat over branches [B, 3*2H]

Mapping onto 6 NeuronCores (task-parallel; batch stays whole because the
recurrent matmul cost is weight-streaming-bound, independent of batch):
  core c in 0..5 handles (branch = c%3, direction = fwd if c<3 else bwd):
    P0: xz_u = x @ Wx_u + b_u                  (dense matmul, M-tiled)
    P1: uni LSTM scan -> transposed h stream hT to DRAM
    P2: xz_d = h1 @ Wx_d + b_d                 (dense matmul over hT tiles;
        written T-REVERSED via indirect-DMA scatter for bwd cores, so one
        SPMD program serves both directions - direction lives in the
        per-core scatter-index table input)
    P3: dir LSTM scan over xz_d with running max -> rmax [64, 512]
Host gathers the 6 rmax outputs into [64, 3072].

Scan step: z (PSUM, [64, 4H]) accumulates xz_t (via identity matmul) plus
h_{t-1} @ Wh (4 K-tile matmuls with the transposed state hT as stationary);
gates on ScalarE from PSUM; c/h updates on VectorE; h re-transposed on PE.
"""
import sys

sys.path.insert(0, "/opt/trn_rl_repo")

import numpy as np

B, T, E, H = 64, 512, 300, 512
FOUR_H = 4 * H
NB = 3

_CACHE = {}


def _build_program(rep=1):
    import concourse.bass as bass
    import concourse.tile as tile
    from concourse import bacc, mybir

    F32 = mybir.dt.float32
    I32 = mybir.dt.int32
    Sig = mybir.ActivationFunctionType.Sigmoid
    Tanh = mybir.ActivationFunctionType.Tanh

    nc = bacc.Bacc("TRN2", target_bir_lowering=False, debug=False,
                   enable_asserts=False, num_devices=6)

    d = {}
    d["xTu"] = nc.dram_tensor("xTu", [T, 128, 3, 64], F32, kind="ExternalInput").ap()
    d["wxu"] = nc.dram_tensor("wxu", [128, 3, FOUR_H], F32, kind="ExternalInput").ap()
    d["whu"] = nc.dram_tensor("whu", [128, 4, FOUR_H], F32, kind="ExternalInput").ap()
    d["bu"] = nc.dram_tensor("bu", [128, FOUR_H], F32, kind="ExternalInput").ap()
    d["wxd"] = nc.dram_tensor("wxd", [128, 4, FOUR_H], F32, kind="ExternalInput").ap()
    d["whd"] = nc.dram_tensor("whd", [128, 4, FOUR_H], F32, kind="ExternalInput").ap()
    d["bd"] = nc.dram_tensor("bd", [128, FOUR_H], F32, kind="ExternalInput").ap()
    d["id64"] = nc.dram_tensor("id64", [64, 64], F32, kind="ExternalInput").ap()
    d["scat"] = nc.dram_tensor("scat", [128, T // 2], I32, kind="ExternalInput").ap()
    d["xzu"] = nc.dram_tensor("xzu", [T, B, FOUR_H], F32, kind="Internal").ap()
    d["hT"] = nc.dram_tensor("hT", [T, 128, 256], F32, kind="Internal").ap()
    d["xzd"] = nc.dram_tensor("xzd", [T, B, FOUR_H], F32, kind="Internal").ap()
    d["rmax"] = nc.dram_tensor("rmax", [B, H], F32, kind="ExternalOutput").ap()

    def build_xproj(tc):
        with (
            tc.tile_pool(name="p0w", bufs=1) as wp,
            tc.tile_pool(name="p0io", bufs=4) as iop,
            tc.tile_pool(name="p0ps", bufs=2, space="PSUM") as psp,
        ):
            wx_sb = wp.tile([128, 3, FOUR_H], F32, tag="wx")
            nc.sync.dma_start(wx_sb[:], d["wxu"])
            b_sb = wp.tile([128, FOUR_H], F32, tag="b")
            nc.sync.dma_start(b_sb[:], d["bu"])
            for m in range(T // 2):
                xt = iop.tile([128, 3, 2, 64], F32, tag="xt")
                nc.sync.dma_start(
                    xt[:], d["xTu"][2 * m:2 * m + 2].rearrange("t p k b -> p k t b"))
                zp = psp.tile([128, FOUR_H], F32, tag="zp")
                for k in range(3):
                    for n in range(4):
                        nc.tensor.matmul(
                            zp[:, bass.ts(n, 512)],
                            xt[:, k, :, :].rearrange("p t b -> p (t b)"),
                            wx_sb[:, k, bass.ts(n, 512)],
                            start=(k == 0), stop=(k == 2))
                zs = iop.tile([128, FOUR_H], F32, tag="zs")
                nc.vector.tensor_add(zs[:], zp[:], b_sb[:])
                nc.sync.dma_start(
                    d["xzu"][2 * m:2 * m + 2].rearrange("t b n -> (t b) n"), zs[:])

    def build_scan(tc, xz, wh_name, store_hT, rmax_out):
        # gate column order is host-permuted to [f i o g]:
        #   chunks: n0=f, n1=i, n2=o, n3=g
        with (
            tc.tile_pool(name=f"w_{wh_name}", bufs=1) as whp,
            tc.tile_pool(name=f"st_{wh_name}", bufs=1) as statep,
            tc.tile_pool(name=f"xz_{wh_name}", bufs=4) as xzp,
            tc.tile_pool(name=f"g_{wh_name}", bufs=2) as gp,
            tc.tile_pool(name=f"zps_{wh_name}", bufs=1, space="PSUM") as zpsp,
            tc.tile_pool(name=f"tps_{wh_name}", bufs=2, space="PSUM") as tpsp,
        ):
            wh_sb = whp.tile([128, 4, FOUR_H], F32, tag="wh")
            nc.sync.dma_start(wh_sb[:], d[wh_name])
            id_sb = whp.tile([64, 64], F32, tag="id")
            nc.sync.dma_start(id_sb[:], d["id64"])

            hT_sb = statep.tile([128, 4, 64], F32, tag="hT")
            # st = [c | tanh(g)] adjacent so one DVE mul makes [f*c | i*tg]
            st_sb = statep.tile([64, 2 * H], F32, tag="st")
            nc.vector.memset(hT_sb[:], 0.0)
            nc.vector.memset(st_sb[:], 0.0)
            if rmax_out is not None:
                rmax_sb = statep.tile([64, H], F32, tag="rmax")
                nc.vector.memset(rmax_sb[:], -1e30)

            for t in range(T):
                xz_t = xzp.tile([64, FOUR_H], F32, tag="xzt")
                nc.sync.dma_start(xz_t[:], xz[t])
                z = zpsp.tile([64, FOUR_H], F32, tag="z")
                for k in range(4):
                    for n in range(4):
                        nc.tensor.matmul(z[:, bass.ts(n, 512)], hT_sb[:, k, :],
                                         wh_sb[:, k, bass.ts(n, 512)],
                                         start=(k == 0), stop=(k == 3))
                zf = gp.tile([64, FOUR_H], F32, tag="zf")
                nc.vector.tensor_add(zf[:], z[:], xz_t[:])
                ga = gp.tile([64, 3 * H], F32, tag="ga")  # [sf si so]
                nc.scalar.activation(ga[:], zf[:, 0:3 * H], Sig)
                nc.scalar.activation(st_sb[:, H:2 * H], zf[:, 3 * H:4 * H], Tanh)
                t12 = gp.tile([64, 2 * H], F32, tag="t12")
                nc.vector.tensor_mul(t12[:], ga[:, 0:2 * H], st_sb[:])
                nc.vector.tensor_add(st_sb[:, 0:H], t12[:, 0:H], t12[:, H:2 * H])
                tc_t = gp.tile([64, H], F32, tag="tc")
                nc.scalar.activation(tc_t[:], st_sb[:, 0:H], Tanh)
                h_t = gp.tile([64, H], F32, tag="h")
                nc.vector.tensor_mul(h_t[:], ga[:, 2 * H:3 * H], tc_t[:])
                if rmax_out is not None:
                    nc.vector.tensor_max(rmax_sb[:], rmax_sb[:], h_t[:])
                pT = tpsp.tile([128, 4, 64], F32, tag="pT")
                for k in range(4):
                    nc.tensor.transpose(pT[:, k, :], h_t[:, bass.ts(k, 128)], id_sb[:])
                nc.vector.tensor_copy(hT_sb[:], pT[:])
                if store_hT:
                    nc.sync.dma_start(d["hT"][t],
                                      hT_sb[:].rearrange("p k b -> p (k b)"))
            if rmax_out is not None:
                nc.sync.dma_start(rmax_out, rmax_sb[:])

    def build_hproj(tc):
        with (
            tc.tile_pool(name="p2w", bufs=1) as wp,
            tc.tile_pool(name="p2io", bufs=4) as iop,
            tc.tile_pool(name="p2ps", bufs=2, space="PSUM") as psp,
        ):
            wx_sb = wp.tile([128, 4, FOUR_H], F32, tag="wx")
            nc.sync.dma_start(wx_sb[:], d["wxd"])
            b_sb = wp.tile([128, FOUR_H], F32, tag="b")
            nc.sync.dma_start(b_sb[:], d["bd"])
            scat_sb = wp.tile([128, T // 2], I32, tag="scat")
            nc.sync.dma_start(scat_sb[:], d["scat"])
            xzd_rows = d["xzd"].rearrange("t b n -> (t b) n")
            for m in range(T // 2):
                ht = iop.tile([128, 4, 2, 64], F32, tag="ht")
                nc.sync.dma_start(
                    ht[:],
                    d["hT"][2 * m:2 * m + 2].rearrange("t p (k b) -> p k t b", k=4))
                zp = psp.tile([128, FOUR_H], F32, tag="zp")
                for k in range(4):
                    for n in range(4):
                        nc.tensor.matmul(
                            zp[:, bass.ts(n, 512)],
                            ht[:, k, :, :].rearrange("p t b -> p (t b)"),
                            wx_sb[:, k, bass.ts(n, 512)],
                            start=(k == 0), stop=(k == 3))
                zs = iop.tile([128, FOUR_H], F32, tag="zs")
                nc.vector.tensor_add(zs[:], zp[:], b_sb[:])
                nc.gpsimd.indirect_dma_start(
                    out=xzd_rows,
                    out_offset=bass.IndirectOffsetOnAxis(
                        ap=scat_sb[:, m:m + 1], axis=0),
                    in_=zs[:],
                    in_offset=None)

    with tile.TileContext(nc) as tc:
        for _ in range(rep):
            build_xproj(tc)
            build_scan(tc, d["xzu"], "whu", store_hT=True, rmax_out=None)
            build_hproj(tc)
            build_scan(tc, d["xzd"], "whd", store_hT=False, rmax_out=d["rmax"])
    nc.compile()
    return nc


def _prep_shared(x):
    """x [B,T,E] -> xT [T,128,3,64] with xT[t,p,k,b] = x[b,t,k*128+p] (E pad 384)."""
    xpad = np.zeros((B, T, 384), np.float32)
    xpad[:, :, :E] = x
    xT = xpad.transpose(1, 2, 0).reshape(T, 3, 128, B).transpose(0, 2, 1, 3)
    return np.ascontiguousarray(xT)


_GATE_PERM = np.r_[H:2 * H, 0:H, 3 * H:4 * H, 2 * H:3 * H]  # [i f g o]->[f i o g]


def _prep_core(xT, wx_u, wh_u, b_u, wx_d, wh_d, b_d, reverse):
    wx_u = np.asarray(wx_u, np.float32)[:, _GATE_PERM]
    wh_u = np.asarray(wh_u, np.float32)[:, _GATE_PERM]
    b_u = np.asarray(b_u, np.float32)[_GATE_PERM]
    wx_d = np.asarray(wx_d, np.float32)[:, _GATE_PERM]
    wh_d = np.asarray(wh_d, np.float32)[:, _GATE_PERM]
    b_d = np.asarray(b_d, np.float32)[_GATE_PERM]
    wxu_pad = np.zeros((384, FOUR_H), np.float32)
    wxu_pad[:E] = wx_u
    p = np.arange(128)
    m = np.arange(T // 2)
    t_src = 2 * m[None, :] + (p[:, None] >= 64)
    t_dst = (T - 1 - t_src) if reverse else t_src
    scat = (t_dst * 64 + (p[:, None] % 64)).astype(np.int32)
    return {
        "xTu": xT,
        "wxu": np.ascontiguousarray(
            wxu_pad.reshape(3, 128, FOUR_H).transpose(1, 0, 2)),
        "whu": np.ascontiguousarray(
            np.asarray(wh_u, np.float32).reshape(4, 128, FOUR_H).transpose(1, 0, 2)),
        "bu": np.ascontiguousarray(
            np.broadcast_to(np.asarray(b_u, np.float32), (128, FOUR_H))),
        "wxd": np.ascontiguousarray(
            np.asarray(wx_d, np.float32).reshape(4, 128, FOUR_H).transpose(1, 0, 2)),
        "whd": np.ascontiguousarray(
            np.asarray(wh_d, np.float32).reshape(4, 128, FOUR_H).transpose(1, 0, 2)),
        "bd": np.ascontiguousarray(
            np.broadcast_to(np.asarray(b_d, np.float32), (128, FOUR_H))),
        "id64": np.eye(64, dtype=np.float32),
        "scat": scat,
    }


def _run(in_maps, rep=1):
    import os
    from concourse.bass_utils import run_bass_kernel_spmd
    key = f"nc{rep}"
    if key not in _CACHE:
        _CACHE[key] = _build_program(rep)
    return run_bass_kernel_spmd(_CACHE[key], in_maps, core_ids=list(range(6)))


def build_in_maps(x, uni_Wx, uni_Wh, uni_b, fwd_Wx, fwd_Wh, fwd_b,
                  bwd_Wx, bwd_Wh, bwd_b):
    xT = _prep_shared(np.asarray(x, np.float32))
    in_maps = []
    for c in range(6):
        br = c % 3
        if c < 3:
            wx_d, wh_d, b_d, rev = fwd_Wx[br], fwd_Wh[br], fwd_b[br], False
        else:
            wx_d, wh_d, b_d, rev = bwd_Wx[br], bwd_Wh[br], bwd_b[br], True
        in_maps.append(_prep_core(xT, np.asarray(uni_Wx[br], np.float32),
                                  uni_Wh[br], uni_b[br], wx_d, wh_d, b_d, rev))
    return in_maps


def kernel(x, uni_Wx, uni_Wh, uni_b, fwd_Wx, fwd_Wh, fwd_b,
           bwd_Wx, bwd_Wh, bwd_b):
    in_maps = build_in_maps(x, uni_Wx, uni_Wh, uni_b, fwd_Wx, fwd_Wh, fwd_b,
                            bwd_Wx, bwd_Wh, bwd_b)
    res = _run(in_maps)
    out = np.empty((B, NB * 2 * H), np.float32)
    for c in range(6):
        br = c % 3
        off = br * 2 * H + (0 if c < 3 else H)
        out[:, off:off + H] = res.results[c]["rmax"]
    return out



# revision 5
# speedup vs baseline: 248.6519x; 248.6519x over previous
"""HBMP (3-branch LSTM + BiLSTM + global max pool) Trainium2 kernel.

Model (B=64, T=512, E=300, H=512, NB=3 branches):
  per branch: h1 = LSTM(x); hf = LSTM(h1); hb = rev(LSTM(rev(h1)))
  emb = maxpool_T(concat([hf, hb], -1));  out = concat over branches [B, 3*2H]

Mapping onto 6 NeuronCores (task-parallel; the recurrent matmul cost is
weight-streaming-bound and independent of batch, so batch stays whole):
  core c handles (branch = c%3, direction = fwd if c<3 else bwd):
    loop1 (For_i, 4 steps/iter): uni LSTM scan with the x-projection
      fused into each step's PSUM accumulation (bias folded into x's
      E-padding as a ones-row); the transposed h stream is stored to
      DRAM in bf16.
    loop2 (For_i, 4 steps/iter): dir LSTM scan with the h1-projection
      fused into each step's PSUM accumulation, consuming the hT stream
      (reversed addressing for bwd cores via partition_id register
      arithmetic - one SPMD program serves both directions) + running max.
Host gathers the 6 rmax outputs into [64, 3072].

Scan step: z (PSUM, [64, 4H]) accumulates x_t(or h1_t) @ Wx (bf16 pairs)
plus h_{t-1} @ Wh (fp32r pairs, 1 cycle/row at N=512); gates on ScalarE
straight from PSUM; c/h updates on VectorE; h re-transposed on PE.
"""
import sys

sys.path.insert(0, "/opt/trn_rl_repo")

import numpy as np
import ml_dtypes

BF16 = ml_dtypes.bfloat16
B, T, E, H = 64, 512, 300, 512
FOUR_H = 4 * H
NB = 3
UNROLL = 4

_CACHE = {}


def _build_program(rep=1):
    import concourse.bass as bass
    import concourse.tile as tile
    from concourse import bacc, mybir

    F32 = mybir.dt.float32
    F32R = mybir.dt.float32r
    BF = mybir.dt.bfloat16
    Sig = mybir.ActivationFunctionType.Sigmoid
    Tanh = mybir.ActivationFunctionType.Tanh
    ds = bass.ds

    nc = bacc.Bacc("TRN2", target_bir_lowering=False, debug=False,
                   enable_asserts=False, num_devices=6)

    d = {}
    # x transposed + padded: xT[t,p,k,b] = xpad[b,t,k*128+p]; row 300 == 1.0
    d["xT"] = nc.dram_tensor("xT", [T, 128, 3, B], BF, kind="ExternalInput").ap()
    # wxu[p,k,:] = Wxu_pad[k*128+p,:] (gate-permuted); row 300 holds uni bias
    d["wxu"] = nc.dram_tensor("wxu", [128, 3, FOUR_H], BF, kind="ExternalInput").ap()
    d["whu"] = nc.dram_tensor("whu", [128, 4, FOUR_H], BF, kind="ExternalInput").ap()
    d["wxd"] = nc.dram_tensor("wxd", [128, 4, FOUR_H], BF, kind="ExternalInput").ap()
    d["whd"] = nc.dram_tensor("whd", [128, 4, FOUR_H], BF, kind="ExternalInput").ap()
    d["bdT"] = nc.dram_tensor("bdT", [1, FOUR_H], BF, kind="ExternalInput").ap()
    d["one1"] = nc.dram_tensor("one1", [1, B], BF, kind="ExternalInput").ap()
    d["id64"] = nc.dram_tensor("id64", [B, B], F32, kind="ExternalInput").ap()
    d["hT"] = nc.dram_tensor("hT", [T, 128, 4 * B], BF, kind="Internal").ap()
    d["rmax"] = nc.dram_tensor("rmax", [B, H], F32, kind="ExternalOutput").ap()

    def build(tc):
        with (
            tc.tile_pool(name="w", bufs=1) as wp,
            tc.tile_pool(name="state", bufs=1) as sp,
            tc.tile_pool(name="io", bufs=2) as iop,
            tc.tile_pool(name="g", bufs=2) as gp,
            tc.tile_pool(name="zps", bufs=1, space="PSUM") as zp,
            tc.tile_pool(name="tps", bufs=2, space="PSUM") as tp,
        ):
            # --- persistent weights ---
            wxu = wp.tile([128, 3, FOUR_H], BF, tag="wxu")
            nc.sync.dma_start(wxu[:], d["wxu"])
            whu = wp.tile([128, 4, FOUR_H], BF, tag="whu")
            nc.sync.dma_start(whu[:], d["whu"])
            wxd = wp.tile([128, 4, FOUR_H], BF, tag="wxd")
            nc.sync.dma_start(wxd[:], d["wxd"])
            whd = wp.tile([128, 4, FOUR_H], BF, tag="whd")
            nc.sync.dma_start(whd[:], d["whd"])
            bdT = wp.tile([1, FOUR_H], BF, tag="bdT")
            nc.sync.dma_start(bdT[:], d["bdT"])
            one1 = wp.tile([1, B], BF, tag="one1")
            nc.sync.dma_start(one1[:], d["one1"])
            id64 = wp.tile([B, B], F32, tag="id64")
            nc.sync.dma_start(id64[:], d["id64"])

            # reversal selector: 0 for cores 0-2 (fwd), 1 for cores 3-5 (bwd)
            s = nc.sync.partition_id() >= 3

            # --- per-scan state ---
            hTs = sp.tile([128, 4, B], BF, tag="hTs")       # transposed h state
            st = sp.tile([B, 2 * H], F32, tag="st")         # [c | tanh g]
            rmax = sp.tile([B, H], F32, tag="rmax")

            def step(xk_stat, wx_sb, wh_sb, delta, bias, hstore, do_rmax):
                """One LSTM step.  xk_stat(k) -> bf16 stationary [128, B]
                for input-projection k-chunk; wx_sb bf16 moving weights
                [128, nk, 4H]; wh_sb fp32 moving recurrent weights."""
                z = zp.tile([B, FOUR_H], F32, tag="z")
                nk = wx_sb.shape[1]
                first = True
                for k in range(nk):
                    xs = xk_stat(k)
                    for n in range(4):
                        nc.tensor.matmul(z[:, bass.ts(n, 512)], xs,
                                         wx_sb[:, k, bass.ts(n, 512)],
                                         start=first, stop=False)
                    first = False
                if bias is not None:
                    for n in range(4):
                        nc.tensor.matmul(z[:, bass.ts(n, 512)], one1[:],
                                         bias[:, bass.ts(n, 512)],
                                         start=False, stop=False)
                for k in range(4):
                    for n in range(4):
                        nc.tensor.matmul(
                            z[:, bass.ts(n, 512)],
                            hTs[:, k, :],
                            wh_sb[:, k, bass.ts(n, 512)],
                            start=False, stop=(k == 3))
                # gates from PSUM; order [f i o g]
                ga = gp.tile([B, 3 * H], F32, tag="ga")
                nc.scalar.activation(ga[:, 0:2 * H], z[:, 0:2 * H], Sig)
                nc.scalar.activation(st[:, H:2 * H], z[:, 3 * H:4 * H], Tanh)
                nc.scalar.activation(ga[:, 2 * H:3 * H], z[:, 2 * H:3 * H], Sig)
                t12 = gp.tile([B, 2 * H], F32, tag="t12")
                nc.vector.tensor_mul(t12[:], ga[:, 0:2 * H], st[:])
                nc.vector.tensor_add(st[:, 0:H], t12[:, 0:H], t12[:, H:2 * H])
                tc_t = gp.tile([B, H], F32, tag="tc")
                nc.scalar.activation(tc_t[:], st[:, 0:H], Tanh)
                h_t = gp.tile([B, H], F32, tag="h")
                nc.vector.tensor_mul(h_t[:], ga[:, 2 * H:3 * H], tc_t[:])
                if do_rmax:
                    nc.vector.tensor_max(rmax[:], rmax[:], h_t[:])
                pT = tp.tile([128, 4, B], F32, tag="pT")
                for k in range(4):
                    nc.tensor.transpose(pT[:, k, :], h_t[:, bass.ts(k, 128)],
                                        id64[:])
                nc.vector.tensor_copy(hTs[:], pT[:])
                if hstore is not None:
                    nc.vector.tensor_copy(hstore[:, delta, :],
                                          pT[:].rearrange("p k b -> p (k b)"))

            # ================= loop 1: uni scan =================
            nc.vector.memset(hTs[:], 0.0)
            nc.vector.memset(st[:], 0.0)
            with tc.For_i(0, T, UNROLL) as i:
                xt = iop.tile([128, UNROLL, 3, B], BF, tag="xt")
                nc.sync.dma_start(
                    xt[:], d["xT"][ds(i, UNROLL)].rearrange("t p k b -> p t k b"))
                hst = iop.tile([128, UNROLL, 4 * B], BF, tag="hst")
                for dt in range(UNROLL):
                    step(lambda k, dt=dt: xt[:, dt, k, :], wxu, whu,
                         dt, None, hst, False)
                nc.sync.dma_start(
                    d["hT"][ds(i, UNROLL)].rearrange("t p e -> p t e"), hst[:])

            # ================= loop 2: dir scan =================
            nc.vector.memset(hTs[:], 0.0)
            nc.vector.memset(st[:], 0.0)
            nc.vector.memset(rmax[:], -1e30)
            with tc.For_i(0, T, UNROLL) as i:
                ht1 = iop.tile([128, UNROLL, 4, B], BF, tag="ht1")
                for dt in range(UNROLL):
                    # fwd: t = i+dt ; bwd: t = (T-1) - (i+dt)
                    tt = i + dt
                    tsrc = nc.s_assert_within(
                        tt + s * (T - 1 - 2 * tt), 0, T - 1,
                        skip_runtime_assert=True)
                    nc.sync.dma_start(
                        ht1[:, dt, :, :].rearrange("p k b -> p (k b)"),
                        d["hT"][ds(tsrc, 1)].rearrange("t p e -> (t p) e"))
                for dt in range(UNROLL):
                    step(lambda k, dt=dt: ht1[:, dt, k, :], wxd, whd,
                         dt, bdT, None, True)
            nc.sync.dma_start(d["rmax"], rmax[:])

    with tile.TileContext(nc) as tc:
        for _ in range(rep):
            build(tc)
    nc.compile()
    return nc


_GATE_PERM = np.r_[H:2 * H, 0:H, 3 * H:4 * H, 2 * H:3 * H]  # [i f g o]->[f i o g]


def _prep_shared(x):
    """x [B,T,E] -> xT [T,128,3,64] bf16 with ones-row at E-index 300."""
    xpad = np.zeros((B, T, 384), np.float32)
    xpad[:, :, :E] = x
    xpad[:, :, E] = 1.0
    xT = xpad.transpose(1, 2, 0).reshape(T, 3, 128, B).transpose(0, 2, 1, 3)
    return np.ascontiguousarray(xT.astype(BF16))


def _prep_core(xT, wx_u, wh_u, b_u, wx_d, wh_d, b_d):
    wx_u = np.asarray(wx_u, np.float32)[:, _GATE_PERM]
    wh_u = np.asarray(wh_u, np.float32)[:, _GATE_PERM]
    b_u = np.asarray(b_u, np.float32)[_GATE_PERM]
    wx_d = np.asarray(wx_d, np.float32)[:, _GATE_PERM]
    wh_d = np.asarray(wh_d, np.float32)[:, _GATE_PERM]
    b_d = np.asarray(b_d, np.float32)[_GATE_PERM]
    wxu_pad = np.zeros((384, FOUR_H), np.float32)
    wxu_pad[:E] = wx_u
    wxu_pad[E] = b_u
    return {
        "xT": xT,
        "wxu": np.ascontiguousarray(
            wxu_pad.reshape(3, 128, FOUR_H).transpose(1, 0, 2).astype(BF16)),
        "whu": np.ascontiguousarray(
            wh_u.reshape(4, 128, FOUR_H).transpose(1, 0, 2).astype(BF16)),
        "wxd": np.ascontiguousarray(
            wx_d.reshape(4, 128, FOUR_H).transpose(1, 0, 2).astype(BF16)),
        "whd": np.ascontiguousarray(
            wh_d.reshape(4, 128, FOUR_H).transpose(1, 0, 2).astype(BF16)),
        "bdT": np.ascontiguousarray(b_d[None, :].astype(BF16)),
        "one1": np.ones((1, B), BF16),
        "id64": np.eye(B, dtype=np.float32),
    }


def _run(in_maps, rep=1):
    from concourse.bass_utils import run_bass_kernel_spmd
    key = f"nc{rep}"
    if key not in _CACHE:
        _CACHE[key] = _build_program(rep)
    return run_bass_kernel_spmd(_CACHE[key], in_maps, core_ids=list(range(6)))


def build_in_maps(x, uni_Wx, uni_Wh, uni_b, fwd_Wx, fwd_Wh, fwd_b,
                  bwd_Wx, bwd_Wh, bwd_b):
    xT = _prep_shared(np.asarray(x, np.float32))
    in_maps = []
    for c in range(6):
        br = c % 3
        if c < 3:
            wx_d, wh_d, b_d = fwd_Wx[br], fwd_Wh[br], fwd_b[br]
        else:
            wx_d, wh_d, b_d = bwd_Wx[br], bwd_Wh[br], bwd_b[br]
        in_maps.append(_prep_core(xT, uni_Wx[br], uni_Wh[br], uni_b[br],
                                  wx_d, wh_d, b_d))
    return in_maps


def kernel(x, uni_Wx, uni_Wh, uni_b, fwd_Wx, fwd_Wh, fwd_b,
           bwd_Wx, bwd_Wh, bwd_b):
    in_maps = build_in_maps(x, uni_Wx, uni_Wh, uni_b, fwd_Wx, fwd_Wh, fwd_b,
                            bwd_Wx, bwd_Wh, bwd_b)
    res = _run(in_maps)
    out = np.empty((B, NB * 2 * H), np.float32)
    for c in range(6):
        br = c % 3
        off = br * 2 * H + (0 if c < 3 else H)
        out[:, off:off + H] = res.results[c]["rmax"]
    return out


# revision 10
# speedup vs baseline: 391.9598x; 1.5763x over previous
"""HBMP (3-branch LSTM + BiLSTM + global max pool) Trainium2 kernel.

Model (B=64, T=512, E=300, H=512, NB=3 branches):
  per branch: h1 = LSTM(x); hf = LSTM(h1); hb = rev(LSTM(rev(h1)))
  emb = maxpool_T(concat([hf, hb], -1));  out = concat over branches [B, 3*2H]

Mapping onto 6 NeuronCores (task-parallel; the recurrent matmul cost is
weight-streaming-bound and independent of batch, so batch stays whole):
  core c handles (branch = c%3, direction = fwd if c<3 else bwd):
    loop1 (For_i, 4 steps/iter): uni LSTM scan with the x-projection
      fused into each step's PSUM accumulation (bias folded into x's
      E-padding as a ones-row); the transposed h stream goes to DRAM bf16.
    loop2 (For_i, 4 steps/iter): dir LSTM scan with the h1-projection
      fused into each step's PSUM accumulation, consuming the hT stream
      (reversed addressing for bwd cores via partition_id register
      arithmetic - one SPMD program serves both directions) + running max.
Host gathers the 6 rmax outputs into [64, 3072].

Step layout ("dup-batch"): gate pre-activations live as two PSUM bank
tiles zb[n] [128, 512] with batch duplicated across partition halves
(lane b+64j holds hidden slice j*256..(j+1)*256); matmuls write the
j=1 half via tile_position=(0,64) column offset with the same [128,64]
stationary (state / x / h1 chunk), so every ScalarE/VectorE op runs on
all 128 lanes with half the per-lane elements.  Bank 0 = gates f|i,
bank 1 = o|g, each closing its own accumulation group so the f|i
sigmoid overlaps the o|g matmuls.  h is re-transposed on PE (4x 64x128
blocks from partition bases 0/64).
"""
import sys

sys.path.insert(0, "/opt/trn_rl_repo")

import numpy as np
import ml_dtypes

BF16 = ml_dtypes.bfloat16
B, T, E, H = 64, 512, 300, 512
FOUR_H = 4 * H
HB = 2 * H  # per-j-half moving width (1024)
NB = 3
UNROLL = 4

_CACHE = {}


def _build_program(rep=1):
    import concourse.bass as bass
    import concourse.tile as tile
    from concourse import bacc, mybir

    F32 = mybir.dt.float32
    BF = mybir.dt.bfloat16
    Sig = mybir.ActivationFunctionType.Sigmoid
    Tanh = mybir.ActivationFunctionType.Tanh
    ds = bass.ds
    Q = H // 2  # 256: per-gate per-j-half column count

    nc = bacc.Bacc("TRN2", target_bir_lowering=False, debug=False,
                   enable_asserts=False, num_devices=6)

    d = {}
    # x transposed + padded: xT[t,p,k,b] = xpad[b,t,k*128+p]; row 300 == 1.0
    d["xT"] = nc.dram_tensor("xT", [T, 128, 3, B], BF, kind="ExternalInput").ap()
    # weights reordered for dup-batch: w[p, k, j, q*Q + c]
    #   = W[k*128+p, gate(q)*H//... see _prep_core]  (gate order f,i,o,g)
    d["wxu"] = nc.dram_tensor("wxu", [128, 3, 2, HB], BF, kind="ExternalInput").ap()
    d["whu"] = nc.dram_tensor("whu", [128, 4, 2, HB], BF, kind="ExternalInput").ap()
    d["wxd"] = nc.dram_tensor("wxd", [128, 4, 2, HB], BF, kind="ExternalInput").ap()
    d["whd"] = nc.dram_tensor("whd", [128, 4, 2, HB], BF, kind="ExternalInput").ap()
    d["bdT"] = nc.dram_tensor("bdT", [1, 2, HB], BF, kind="ExternalInput").ap()
    d["one1"] = nc.dram_tensor("one1", [1, B], BF, kind="ExternalInput").ap()
    d["id64"] = nc.dram_tensor("id64", [B, B], F32, kind="ExternalInput").ap()
    d["hT"] = nc.dram_tensor("hT", [T, 128, 4 * B], BF, kind="Internal").ap()
    d["rmax"] = nc.dram_tensor("rmax", [B, H], F32, kind="ExternalOutput").ap()

    def build(tc):
        with (
            tc.tile_pool(name="w", bufs=1) as wp,
            tc.tile_pool(name="state", bufs=1) as sp,
            tc.tile_pool(name="io", bufs=2) as iop,
            tc.tile_pool(name="g", bufs=2) as gp,
            tc.tile_pool(name="zps", bufs=2, space="PSUM") as zp,
            tc.tile_pool(name="tps", bufs=2, space="PSUM") as tp,
        ):
            # --- persistent weights ---
            wxu = wp.tile([128, 3, 2, HB], BF, tag="wxu")
            nc.sync.dma_start(wxu[:], d["wxu"])
            whu = wp.tile([128, 4, 2, HB], BF, tag="whu")
            nc.sync.dma_start(whu[:], d["whu"])
            wxd = wp.tile([128, 4, 2, HB], BF, tag="wxd")
            nc.sync.dma_start(wxd[:], d["wxd"])
            whd = wp.tile([128, 4, 2, HB], BF, tag="whd")
            nc.sync.dma_start(whd[:], d["whd"])
            bdT = wp.tile([1, 2, HB], BF, tag="bdT")
            nc.sync.dma_start(bdT[:], d["bdT"])
            one1 = wp.tile([1, B], BF, tag="one1")
            nc.sync.dma_start(one1[:], d["one1"])
            id64 = wp.tile([B, B], F32, tag="id64")
            nc.sync.dma_start(id64[:], d["id64"])

            # reversal selector: 0 for cores 0-2 (fwd), 1 for cores 3-5 (bwd)
            s = nc.sync.partition_id() >= 3

            # --- per-scan state ---
            hTs = sp.tile([128, 4, B], BF, tag="hTs")       # transposed h state
            st = sp.tile([128, 2 * Q], F32, tag="st")       # [c | tanh g]
            rmax = sp.tile([B, H], F32, tag="rmax")

            def step(xk_stat, nk, wx_sb, wh_sb, delta, bias, hstore, do_rmax):
                """One LSTM step in dup-batch layout.
                xk_stat(k) -> bf16 stationary [128, B] for the input-
                projection k-chunk (x_t or h1_t); wx/wh moving [128,nk,2,HB].
                Bank n=0 holds gates f|i, n=1 holds o|g (Q cols each)."""
                zb = [zp.tile([128, 512], F32, tag=f"zb{n}", name=f"zb{n}")
                      for n in range(2)]
                # Each (bank, j-half) quadrant is its own PSUM accumulation
                # group (the has_written clear covers the full bank width but
                # only the addressed partitions).  j=0/j=1 pairs are issued
                # back-to-back: they hit disjoint PE column groups, so their
                # moving streams overlap on different sub-arrays.
                # input-projection (+bias) matmuls: independent of the
                # recurrent state, scheduled into the previous step's gaps
                TP = (None, (0, 64))
                for n in range(2):
                    for k in range(nk):
                        for j in range(2):
                            nc.tensor.matmul(
                                zb[n][64 * j:64 * j + 64, :], xk_stat(k),
                                wx_sb[:, k, j, bass.ts(n, 512)],
                                start=(k == 0), stop=False, tile_position=TP[j])
                    if bias is not None:
                        for j in range(2):
                            nc.tensor.matmul(
                                zb[n][64 * j:64 * j + 64, :], one1[:],
                                bias[:, j, bass.ts(n, 512)],
                                start=False, stop=False, tile_position=TP[j])
                # recurrent matmuls; bank n closes its own groups so the
                # f|i sigmoid overlaps bank 1's matmuls
                for n in range(2):
                    for k in range(4):
                        for j in range(2):
                            nc.tensor.matmul(
                                zb[n][64 * j:64 * j + 64, :], hTs[:, k, :],
                                wh_sb[:, k, j, bass.ts(n, 512)],
                                start=False, stop=(k == 3), tile_position=TP[j])
                # gates: zb0 = [f|i], zb1 = [o|g]
                ga = gp.tile([128, 2 * Q], F32, tag="ga")   # [sf | si]
                go = gp.tile([128, Q], F32, tag="go")       # so
                nc.scalar.activation(ga[:], zb[0][:], Sig)
                nc.scalar.activation(st[:, Q:2 * Q], zb[1][:, Q:2 * Q], Tanh)
                nc.scalar.activation(go[:], zb[1][:, 0:Q], Sig)
                t12 = gp.tile([128, 2 * Q], F32, tag="t12")
                nc.vector.tensor_mul(t12[:], ga[:], st[:])
                nc.vector.tensor_add(st[:, 0:Q], t12[:, 0:Q], t12[:, Q:2 * Q])
                tc_t = gp.tile([128, Q], F32, tag="tc")
                nc.scalar.activation(tc_t[:], st[:, 0:Q], Tanh)
                # h in flat [64, 512] layout via two cross-base muls
                # (hf[b, j*256+c] = h[b, hid]); transposes then run from
                # partition base 0 (base-64 transposes hang the device)
                hf = gp.tile([B, H], F32, tag="hf")
                for j in range(2):
                    nc.vector.tensor_mul(hf[:, bass.ts(j, Q)],
                                         go[64 * j:64 * j + 64, :],
                                         tc_t[64 * j:64 * j + 64, :])
                if do_rmax:
                    nc.vector.tensor_max(rmax[:], rmax[:], hf[:])
                pT = tp.tile([128, 4, B], F32, tag="pT")
                for k in range(4):
                    nc.tensor.transpose(pT[:, k, :], hf[:, bass.ts(k, 128)],
                                        id64[:])
                nc.vector.tensor_copy(hTs[:], pT[:])
                if hstore is not None:
                    nc.vector.tensor_copy(hstore[:, delta, :],
                                          pT[:].rearrange("p k b -> p (k b)"))

            # ================= loop 1: uni scan =================
            nc.vector.memset(hTs[:], 0.0)
            nc.vector.memset(st[:], 0.0)
            with tc.For_i(0, T, UNROLL) as i:
                xt = iop.tile([128, UNROLL, 3, B], BF, tag="xt")
                nc.sync.dma_start(
                    xt[:], d["xT"][ds(i, UNROLL)].rearrange("t p k b -> p t k b"))
                hst = iop.tile([128, UNROLL, 4 * B], BF, tag="hst")
                for dt in range(UNROLL):
                    step(lambda k, dt=dt: xt[:, dt, k, :], 3, wxu, whu,
                         dt, None, hst, False)
                nc.sync.dma_start(
                    d["hT"][ds(i, UNROLL)].rearrange("t p e -> p t e"), hst[:])

            # ================= loop 2: dir scan =================
            nc.vector.memset(hTs[:], 0.0)
            nc.vector.memset(st[:], 0.0)
            nc.vector.memset(rmax[:], -1e30)
            with tc.For_i(0, T, UNROLL) as i:
                ht1 = iop.tile([128, UNROLL, 4, B], BF, tag="ht1")
                for dt in range(UNROLL):
                    # fwd: t = i+dt ; bwd: t = (T-1) - (i+dt)
                    tt = i + dt
                    tsrc = nc.s_assert_within(
                        tt + s * (T - 1 - 2 * tt), 0, T - 1,
                        skip_runtime_assert=True)
                    nc.sync.dma_start(
                        ht1[:, dt, :, :].rearrange("p k b -> p (k b)"),
                        d["hT"][ds(tsrc, 1)].rearrange("t p e -> (t p) e"))
                for dt in range(UNROLL):
                    step(lambda k, dt=dt: ht1[:, dt, k, :], 4, wxd, whd,
                         dt, bdT, None, True)
            nc.sync.dma_start(d["rmax"], rmax[:])

    with tile.TileContext(nc) as tc:
        for _ in range(rep):
            build(tc)
    nc.compile()
    return nc


_GATE_PERM = np.r_[H:2 * H, 0:H, 3 * H:4 * H, 2 * H:3 * H]  # [i f g o]->[f i o g]


def _dup_cols(w):
    """[rows, 4H] gate-ordered [f i o g] -> [rows, 2j, (n,q',c)=HB] so that
    moving slice w2[:, j, n*512:(n+1)*512] covers gates (2n, 2n+1), j-half."""
    rows = w.shape[0]
    # w[r, q*H + j*256 + c] -> w2[r, j, q*256 + c]
    w5 = w.reshape(rows, 4, 2, 256)            # [r, q, j, c]
    return np.ascontiguousarray(w5.transpose(0, 2, 1, 3).reshape(rows, 2, HB))


def _prep_shared(x):
    """x [B,T,E] -> xT [T,128,3,64] bf16 with ones-row at E-index 300."""
    xpad = np.zeros((B, T, 384), np.float32)
    xpad[:, :, :E] = x
    xpad[:, :, E] = 1.0
    xT = xpad.transpose(1, 2, 0).reshape(T, 3, 128, B).transpose(0, 2, 1, 3)
    return np.ascontiguousarray(xT.astype(BF16))


def _chunk(w, nk):
    """[nk*128, 2, HB] -> [128, nk, 2, HB] bf16."""
    return np.ascontiguousarray(
        w.reshape(nk, 128, 2, HB).transpose(1, 0, 2, 3).astype(BF16))


def _prep_core(xT, wx_u, wh_u, b_u, wx_d, wh_d, b_d):
    wx_u = np.asarray(wx_u, np.float32)[:, _GATE_PERM]
    wh_u = np.asarray(wh_u, np.float32)[:, _GATE_PERM]
    b_u = np.asarray(b_u, np.float32)[_GATE_PERM]
    wx_d = np.asarray(wx_d, np.float32)[:, _GATE_PERM]
    wh_d = np.asarray(wh_d, np.float32)[:, _GATE_PERM]
    b_d = np.asarray(b_d, np.float32)[_GATE_PERM]
    wxu_pad = np.zeros((384, FOUR_H), np.float32)
    wxu_pad[:E] = wx_u
    wxu_pad[E] = b_u
    return {
        "xT": xT,
        "wxu": _chunk(_dup_cols(wxu_pad), 3),
        "whu": _chunk(_dup_cols(wh_u), 4),
        "wxd": _chunk(_dup_cols(wx_d), 4),
        "whd": _chunk(_dup_cols(wh_d), 4),
        "bdT": np.ascontiguousarray(_dup_cols(b_d[None, :]).astype(BF16)),
        "one1": np.ones((1, B), BF16),
        "id64": np.eye(B, dtype=np.float32),
    }


def _run(in_maps, rep=1):
    from concourse.bass_utils import run_bass_kernel_spmd
    key = f"nc{rep}"
    if key not in _CACHE:
        _CACHE[key] = _build_program(rep)
    return run_bass_kernel_spmd(_CACHE[key], in_maps, core_ids=list(range(6)))


def build_in_maps(x, uni_Wx, uni_Wh, uni_b, fwd_Wx, fwd_Wh, fwd_b,
                  bwd_Wx, bwd_Wh, bwd_b):
    xT = _prep_shared(np.asarray(x, np.float32))
    in_maps = []
    for c in range(6):
        br = c % 3
        if c < 3:
            wx_d, wh_d, b_d = fwd_Wx[br], fwd_Wh[br], fwd_b[br]
        else:
            wx_d, wh_d, b_d = bwd_Wx[br], bwd_Wh[br], bwd_b[br]
        in_maps.append(_prep_core(xT, uni_Wx[br], uni_Wh[br], uni_b[br],
                                  wx_d, wh_d, b_d))
    return in_maps


def kernel(x, uni_Wx, uni_Wh, uni_b, fwd_Wx, fwd_Wh, fwd_b,
           bwd_Wx, bwd_Wh, bwd_b):
    in_maps = build_in_maps(x, uni_Wx, uni_Wh, uni_b, fwd_Wx, fwd_Wh, fwd_b,
                            bwd_Wx, bwd_Wh, bwd_b)
    res = _run(in_maps)
    out = np.empty((B, NB * 2 * H), np.float32)
    for c in range(6):
        br = c % 3
        off = br * 2 * H + (0 if c < 3 else H)
        out[:, off:off + H] = res.results[c]["rmax"]
    return out
